# revision 1
# baseline (speedup 1.0000x reference)
"""Trainium2 Bass kernel for AdjStackAttentionWeights.

reference:  out = einsum('bsij,hs->bhij', stacks, W) + b[None,:,None,None]
            out = where(mask[:,None,:,:], 0.0, out)
shapes:     stacks [16,16,512,512] f32, mask [16,512,512] bool,
            W [8,16] f32, b [8] f32  ->  out [16,8,512,512] f32

Data-parallel over batch: 2 graphs per core x 8 cores.

The host shards AND re-lays-out the inputs into the exact on-chip tile
layouts so every DMA is fully contiguous (strided s-gather reads cap at
~200GB/s on TRN2 vs ~355GB/s contiguous; same HBM bytes either way).
The boolean mask is pre-broadcast over h on the host (uint8) so masking
is a plain elementwise multiply -- no broadcast matmuls on-chip.

Per graph, i in 4 superblocks w of 128 rows; i = 128w + 16*ih + il,
il = 8*c1 + i_in (c1 in {0,1}, i_in in [0,8)); cd = 2*ih + c1:

  rhs tile  [128,8192] f32 per (b,w): p = 8s+ih, f = il*512+j
      (one fully contiguous 4MB DMA, alternating the two HWDGE rings
       so two reads stay in flight and HBM latency spikes are hidden)
  keep tile [128,4096] u8 per (b,w): p = 8cd+h, f = i_in*512+j (512KB)
  psum [128,512] per (w,i_in): p = 8cd+h; two zero-padded-lhsT matmuls
      accumulate (c1=0,1): lhsT w_bd[8s+ih, 128c1 + 8(2ih+c1)+h] = W[h,s]
  epilogue: one DVE op: out = (psum + bias) * keep
  out tile [128,4096] f32 per (b,w): p = 8cd+h, f = i_in*512+j
      (one 2MB DMA on the SWDGE ring; 16KB h-strided runs write at
       line rate and writes tolerate the SWDGE issue latency)
Matmuls run as float32r so no input cast is needed (measured rel err
~1.4e-4 vs the f32 reference; the PE streams ~1 col/ns either dtype).
"""

import numpy as np
import ml_dtypes

B, S, N, H = 16, 16, 512, 8
NCORES = 8
BPC = B // NCORES  # graphs per core

MODE = "f32r"  # "f32r" | "bf16"

_CACHE = {}


def _build():
    import concourse.bacc as bacc
    import concourse.mybir as mybir
    import concourse.tile as tile

    f32 = mybir.dt.float32
    bf16 = mybir.dt.bfloat16
    cdt = mybir.dt.float32r if MODE == "f32r" else bf16

    nc = bacc.Bacc("TRN2", target_bir_lowering=False, debug=False,
                   num_devices=NCORES)

    # host-relaid stacks: [b, w, p=8s+ih, f=il*512+j]
    srl = nc.dram_tensor("srl", [BPC, 4, 128, 8192],
                         cdt if MODE == "f32r" else f32,
                         kind="ExternalInput")
    # host-broadcast keep mask: [b, w, p=8cd+h, f=i_in*512+j] uint8
    krl = nc.dram_tensor("krl", [BPC, 4, 128, 4096], mybir.dt.uint8,
                         kind="ExternalInput")
    w_bd = nc.dram_tensor("w_bd", [128, 256], cdt, kind="ExternalInput")
    bias = nc.dram_tensor("bias", [128, 1], f32, kind="ExternalInput")
    out = nc.dram_tensor("out", [BPC, H, N, N], f32, kind="ExternalOutput")

    # out per (b, w): [cd(16), h(8), (i_in j)(4096)]
    oview2 = out.ap().rearrange("b h (w cd iin) j -> b w cd h (iin j)",
                                w=4, cd=16, iin=8)

    ADD = mybir.AluOpType.add
    MULT = mybir.AluOpType.mult

    with tile.TileContext(nc) as tc:
        with (
            tc.tile_pool(name="const", bufs=1) as cpool,
            tc.tile_pool(name="maskp", bufs=2) as mpool,
            tc.tile_pool(name="data", bufs=4) as dpool,
            tc.tile_pool(name="outp", bufs=2) as opool,
            tc.tile_pool(name="psd", bufs=8, space="PSUM") as psd_pool,
        ):
            wbd_t = cpool.tile([128, 256], cdt)
            nc.sync.dma_start(wbd_t[:], w_bd.ap())
            bias_t = cpool.tile([128, 1], f32)
            nc.sync.dma_start(bias_t[:], bias.ap())

            for bb in range(BPC):
                for w in range(4):
                    rhs_t = dpool.tile([128, 8192], cdt, tag="rhs")
                    reng = nc.sync if (bb * 4 + w) % 2 == 0 else nc.scalar
                    if bb == 0 and w == 0:
                        # chunked first load in c1-paired order so the
                        # first psums unblock after ~2MB instead of 4MB
                        for fsl in (0, 4096, 2048, 6144):
                            reng.dma_start(
                                rhs_t[:, fsl:fsl + 2048],
                                srl.ap()[bb, w][:, fsl:fsl + 2048])
                    else:
                        reng.dma_start(rhs_t[:], srl.ap()[bb, w])
                    mask_t = mpool.tile([128, 4096], mybir.dt.uint8, tag="mask")
                    nc.sync.dma_start(mask_t[:], krl.ap()[bb, w])
                    out_t = opool.tile([128, 4096], f32)
                    for i_in in range(8):
                        ps_d = psd_pool.tile([128, 512], f32)
                        for c1 in range(2):
                            fsl = (8 * c1 + i_in) * 512
                            nc.tensor.matmul(
                                ps_d[:, :],
                                wbd_t[:, c1 * 128:c1 * 128 + 128],
                                rhs_t[:, fsl:fsl + 512],
                                start=(c1 == 0), stop=(c1 == 1))
                        # out = (ps_d + bias) * keep
                        nc.vector.scalar_tensor_tensor(
                            out_t[:, i_in * 512:i_in * 512 + 512], ps_d[:],
                            bias_t[:],
                            mask_t[:, i_in * 512:i_in * 512 + 512],
                            op0=ADD, op1=MULT)
                    nc.gpsimd.dma_start(oview2[bb, w], out_t[:])

    nc.compile()
    return nc


def _prep_consts(W, b):
    # lhsT for the c1-th accumulating matmul lives in w_bd[:, 128*c1:...]
    # w_bd[8s+ih, 128*c1 + 8*(2ih+c1) + h] = W[h, s]; rest zero.
    w_bd = np.zeros((128, 256), dtype=np.float32)
    for c1 in range(2):
        for ih in range(8):
            base = 128 * c1 + 8 * (2 * ih + c1)
            for h in range(8):
                w_bd[ih::8, base + h] = W[h, :]  # rows k = 8s+ih
    bias = np.tile(np.asarray(b, np.float32), 16).reshape(128, 1)
    if MODE == "bf16":
        w_bd = w_bd.astype(ml_dtypes.bfloat16)
    return w_bd, bias


def _relayout(stacks, mask):
    # srl[b, w, 8s+ih, il*512+j] = stacks[b, s, 128w+16ih+il, j]
    srl = stacks.reshape(B, S, 4, 8, 16, N)          # b s w ih il j
    srl = np.ascontiguousarray(srl.transpose(0, 2, 1, 3, 4, 5))
    srl = srl.reshape(B, 4, 128, 8192)
    # krl[b, w, 8cd+h, i_in*512+j] = 1 - mask[b, 128w+8cd+i_in, j]
    keep = (~np.asarray(mask, bool)).reshape(B, 4, 16, 8, N)  # b w cd iin j
    krl = np.broadcast_to(keep[:, :, :, None, :, :],
                          (B, 4, 16, 8, 8, N))                # b w cd h iin j
    krl = np.ascontiguousarray(krl.astype(np.uint8))
    krl = krl.reshape(B, 4, 128, 4096)
    return srl, krl


def kernel(stacks, mask, W, b):
    from concourse.bass_utils import run_bass_kernel_spmd

    if "nc" not in _CACHE:
        _CACHE["nc"] = _build()
    nc = _CACHE["nc"]

    stacks = np.asarray(stacks, dtype=np.float32)
    srl, krl = _relayout(stacks, np.asarray(mask))
    w_bd, bias = _prep_consts(np.asarray(W, np.float32),
                              np.asarray(b, np.float32))

    in_maps = []
    for c in range(NCORES):
        in_maps.append({
            "srl": srl[c * BPC:(c + 1) * BPC],
            "krl": krl[c * BPC:(c + 1) * BPC],
            "w_bd": w_bd, "bias": bias,
        })

    res = run_bass_kernel_spmd(nc, in_maps, core_ids=list(range(NCORES)),
                               **_CACHE.get("run_kwargs", {}))
    _CACHE["last_result"] = res
    outs = [r["out"] for r in res.results]
    return np.concatenate(outs, axis=0)



# revision 4
# speedup vs baseline: 1.6591x; 1.6591x over previous
"""Trainium2 Bass kernel for AdjStackAttentionWeights.

reference:  out = einsum('bsij,hs->bhij', stacks, W) + b[None,:,None,None]
            out = where(mask[:,None,:,:], 0.0, out)
shapes:     stacks [16,16,512,512] f32, mask [16,512,512] bool,
            W [8,16] f32, b [8] f32  ->  out [16,8,512,512] f32

Data-parallel over batch: 2 graphs per core x 8 cores.

The kernel is DMA-fabric-bound (16 engines x 22.5 B/ns = 360 GB/s per
core), so the host re-lays-out AND compresses the streams to the
minimum byte count; rel-err budget is 2e-2 so bf16 I/O is safe:

  srl  [2,4,128,8192] bf16 (16 MB/core): stacks pre-masked (masked
       pairs zeroed) and pre-transposed so every DMA is contiguous.
  krl  [128,4096] u8 (0.5 MB/core): keep mask, NOT pre-broadcast over
       h; one contiguous load, converted once to bf16 on-chip.
  out  [2,8,512,512] bf16 (8 MB/core), host upcasts to f32.

Per graph, i in 4 superblocks w of 128 rows; i = 128w + 16*ih + il,
il = 8*c1 + i_in (c1 in {0,1}, i_in in [0,8)); cd = 2*ih + c1, so
i = 128w + 8cd + i_in.  Out/psum partition p = 16h + cd.

  rhs tile [128,8192] bf16 per (b,w): p = 8s+ih, f = il*512+j
  psum [128,512] per (w,i_in): THREE accumulating matmuls --
    c1=0,1: lhsT w_bd[8s+ih, 128c1 + 16h+2ih+c1] = W[h,s]
            (zero-padded block-diagonal routing, 512 rhs cols each)
    bias:   lhsT blhs[16t+cd, 128t + 16h+cd] = b[h] against the bf16
            keep tile -- adds b[h]*keep[i,j], so masked pairs stay
            exactly 0 and no mask broadcast / epilogue multiply exists
  epilogue: one DVE copy psum f32 -> out tile bf16
  out tile [128,4096] bf16 per (b,w): p = 16h+cd, f = i_in*512+j
      (8 KB contiguous runs per partition on the SWDGE ring)
"""

import numpy as np
import ml_dtypes

B, S, N, H = 16, 16, 512, 8
NCORES = 8
BPC = B // NCORES  # graphs per core

_CACHE = {}


def _build():
    import concourse.bacc as bacc
    import concourse.mybir as mybir
    import concourse.tile as tile

    f32 = mybir.dt.float32
    bf16 = mybir.dt.bfloat16

    nc = bacc.Bacc("TRN2", target_bir_lowering=False, debug=False,
                   num_devices=NCORES)

    # host-relaid, pre-masked stacks: [b, w, p=8s+ih, f=il*512+j]
    srl = nc.dram_tensor("srl", [BPC, 4, 128, 8192], bf16,
                         kind="ExternalInput")
    # keep mask (no h-broadcast): [p=16*(4b+w)+cd, f=i_in*512+j]
    krl = nc.dram_tensor("krl", [128, 4096], mybir.dt.uint8,
                         kind="ExternalInput")
    w_bd = nc.dram_tensor("w_bd", [128, 256], bf16, kind="ExternalInput")
    blhs = nc.dram_tensor("blhs", [128, 1024], bf16, kind="ExternalInput")
    out = nc.dram_tensor("out", [BPC, H, N, N], bf16, kind="ExternalOutput")

    # out per (b, w): [h(8), cd(16), (i_in j)(4096)] -> p = 16h+cd
    oview = out.ap().rearrange("b h (w cd iin) j -> b w h cd (iin j)",
                               w=4, cd=16, iin=8)

    with tile.TileContext(nc) as tc:
        with (
            tc.tile_pool(name="const", bufs=1) as cpool,
            tc.tile_pool(name="data", bufs=4) as dpool,
            tc.tile_pool(name="outp", bufs=2) as opool,
            tc.tile_pool(name="psd", bufs=8, space="PSUM") as psd_pool,
        ):
            wbd_t = cpool.tile([128, 256], bf16)
            nc.sync.dma_start(wbd_t[:], w_bd.ap())
            blhs_t = cpool.tile([128, 1024], bf16)
            nc.sync.dma_start(blhs_t[:], blhs.ap())
            krl_t = cpool.tile([128, 4096], mybir.dt.uint8)
            nc.sync.dma_start(krl_t[:], krl.ap())
            # one-time u8 -> bf16 convert; feeds the bias matmuls
            krl_bf = cpool.tile([128, 4096], bf16)
            nc.vector.tensor_copy(krl_bf[:], krl_t[:])

            for bb in range(BPC):
                for w in range(4):
                    t = 4 * bb + w
                    rhs_t = dpool.tile([128, 8192], bf16, tag="rhs")
                    reng = nc.sync if t % 2 == 0 else nc.scalar
                    if t == 0:
                        # chunked first load in c1-paired order so the
                        # first psums unblock after ~1MB instead of 2MB
                        for fsl in (0, 4096, 2048, 6144):
                            reng.dma_start(
                                rhs_t[:, fsl:fsl + 2048],
                                srl.ap()[bb, w][:, fsl:fsl + 2048])
                    else:
                        reng.dma_start(rhs_t[:], srl.ap()[bb, w])
                    out_t = opool.tile([128, 4096], bf16)
                    for i_in in range(8):
                        ps = psd_pool.tile([128, 512], f32)
                        for c1 in range(2):
                            fsl = (8 * c1 + i_in) * 512
                            nc.tensor.matmul(
                                ps[:, :],
                                wbd_t[:, c1 * 128:c1 * 128 + 128],
                                rhs_t[:, fsl:fsl + 512],
                                start=(c1 == 0), stop=False)
                        nc.tensor.matmul(
                            ps[:, :],
                            blhs_t[:, t * 128:t * 128 + 128],
                            krl_bf[:, i_in * 512:i_in * 512 + 512],
                            start=False, stop=True)
                        nc.vector.tensor_copy(
                            out_t[:, i_in * 512:i_in * 512 + 512], ps[:])
                    nc.gpsimd.dma_start(oview[bb, w], out_t[:])

    nc.compile()
    return nc


def _prep_consts(W, b):
    # c1-th accumulating matmul lhsT in w_bd[:, 128*c1:...]:
    # w_bd[8s+ih, 128*c1 + 16h + 2ih + c1] = W[h, s]; rest zero.
    w_bd = np.zeros((128, 256), dtype=np.float32)
    for c1 in range(2):
        for ih in range(8):
            for h in range(8):
                m = 16 * h + 2 * ih + c1
                w_bd[ih::8, 128 * c1 + m] = W[h, :]  # rows k = 8s+ih
    # bias-keep lhsT per tile t: blhs[16t+cd, 128t + 16h+cd] = b[h]
    blhs = np.zeros((128, 1024), dtype=np.float32)
    for t in range(8):
        for cd in range(16):
            for h in range(8):
                blhs[16 * t + cd, 128 * t + 16 * h + cd] = b[h]
    return (w_bd.astype(ml_dtypes.bfloat16),
            blhs.astype(ml_dtypes.bfloat16))


def _relayout(stacks, mask):
    keep = ~np.asarray(mask, bool)                       # [B, N, N]
    # pre-mask: masked pairs contribute exactly 0 to every h
    sm = np.asarray(stacks, np.float32) * keep[:, None, :, :]
    # srl[b, w, 8s+ih, il*512+j] = sm[b, s, 128w+16ih+il, j]
    srl = sm.reshape(B, S, 4, 8, 16, N)                  # b s w ih il j
    srl = srl.transpose(0, 2, 1, 3, 4, 5)                # b w s ih il j
    srl = np.ascontiguousarray(srl, dtype=ml_dtypes.bfloat16)
    srl = srl.reshape(B, 4, 128, 8192)
    # krl[16*(4b'+w)+cd, iin*512+j] = keep[b, 128w+8cd+iin, j], per core
    krl = keep.astype(np.uint8).reshape(B, 4, 16, 8, N)  # b w cd iin j
    krl = krl.reshape(NCORES, BPC * 4 * 16, 8 * N)       # core, 128, 4096
    return srl, krl


def kernel(stacks, mask, W, b):
    from concourse.bass_utils import run_bass_kernel_spmd

    if "nc" not in _CACHE:
        _CACHE["nc"] = _build()
    nc = _CACHE["nc"]

    srl, krl = _relayout(stacks, mask)
    w_bd, blhs = _prep_consts(np.asarray(W, np.float32),
                              np.asarray(b, np.float32))

    in_maps = []
    for c in range(NCORES):
        in_maps.append({
            "srl": srl[c * BPC:(c + 1) * BPC],
            "krl": krl[c],
            "w_bd": w_bd, "blhs": blhs,
        })

    res = run_bass_kernel_spmd(nc, in_maps, core_ids=list(range(NCORES)),
                               **_CACHE.get("run_kwargs", {}))
    _CACHE["last_result"] = res
    outs = [np.asarray(r["out"]) for r in res.results]
    return np.concatenate(outs, axis=0).astype(np.float32)


# revision 8
# speedup vs baseline: 1.7450x; 1.0518x over previous
"""Trainium2 Bass kernel for AdjStackAttentionWeights.

reference:  out = einsum('bsij,hs->bhij', stacks, W) + b[None,:,None,None]
            out = where(mask[:,None,:,:], 0.0, out)
shapes:     stacks [16,16,512,512] f32, mask [16,512,512] bool,
            W [8,16] f32, b [8] f32  ->  out [16,8,512,512] f32

Data-parallel over batch: 2 graphs per core x 8 cores.

The kernel is DMA-fabric-bound (16 engines x 22.5 B/ns = 360 GB/s per
core), so the host re-lays-out AND compresses the streams to the
minimum byte count; rel-err budget is 2e-2 so bf16 I/O is safe:

  srl  [2,4,128,8192] bf16 (16 MB/core): stacks pre-masked (masked
       pairs zeroed) and pre-transposed so every DMA is contiguous.
  krl  [128,4096] u8 (0.5 MB/core): keep mask, NOT pre-broadcast over
       h; one contiguous load, converted once to bf16 on-chip.
  out  [2,8,512,512] bf16 (8 MB/core), host upcasts to f32.

Per graph, i in 4 superblocks w of 128 rows; i = 128w + 16*ih + il,
il = 8*c1 + i_in (c1 in {0,1}, i_in in [0,8)); cd = 2*ih + c1, so
i = 128w + 8cd + i_in.  Out/psum partition p = 16h + cd.

  rhs tile [128,8192] bf16 per (b,w): p = 8s+ih, f = il*512+j
  psum [128,1024] (2 banks) per (w, i_in pair): per i_in THREE
  accumulating matmuls --
    c1=0,1: lhsT w_bd[8s+ih, 128c1 + 16h+2ih+c1] = W[h,s]
            (zero-padded block-diagonal routing, 512 rhs cols each)
    bias:   lhsT blhs[16t+cd, 128t + 16h+cd] = b[h] against the bf16
            keep tile -- adds b[h]*keep[i,j], so masked pairs stay
            exactly 0 and no mask broadcast / epilogue multiply exists
  epilogue: 1024-wide copies psum f32 -> out tile bf16, alternating
    Vector / Activation engines so neither serializes the psum drain
  out tile [128,4096] bf16 per (b,w), written as two 2048-wide halves
      on the SWDGE ring as soon as each half's copies land (4 KB
      contiguous runs per partition)

Schedule notes (from perfetto traces): all read dma_starts issue
up-front so queue order never trails compute; consts + krl go on the
otherwise-idle SWDGE ring so the two HWDGE read rings boot straight
into stacks data; tile 0 is loaded as 4 independent chunk tiles split
across both read rings so the first matmuls unblock ~6us earlier.
"""

import numpy as np
import ml_dtypes

B, S, N, H = 16, 16, 512, 8
NCORES = 8
BPC = B // NCORES  # graphs per core

_CACHE = {}


def _build():
    import concourse.bacc as bacc
    import concourse.mybir as mybir
    import concourse.tile as tile

    f32 = mybir.dt.float32
    bf16 = mybir.dt.bfloat16

    nc = bacc.Bacc("TRN2", target_bir_lowering=False, debug=False,
                   num_devices=NCORES)

    srl = nc.dram_tensor("srl", [BPC, 4, 128, 8192], bf16,
                         kind="ExternalInput")
    krl = nc.dram_tensor("krl", [128, 4096], mybir.dt.uint8,
                         kind="ExternalInput")
    w_bd = nc.dram_tensor("w_bd", [128, 256], bf16, kind="ExternalInput")
    blhs = nc.dram_tensor("blhs", [128, 1024], bf16, kind="ExternalInput")
    out = nc.dram_tensor("out", [BPC, H, N, N], bf16, kind="ExternalOutput")

    # out halves per (b, w, c): p = 16h+cd, f = i_in*512+j, i_in in
    # [4c, 4c+4) -> 4 KB contiguous DRAM runs per partition
    oview = out.ap().rearrange("b h (w cd c iin) j -> b w c h cd (iin j)",
                               w=4, cd=16, c=2, iin=4)

    with tile.TileContext(nc) as tc:
        with (
            tc.tile_pool(name="const", bufs=1) as cpool,
            tc.tile_pool(name="chunk", bufs=4) as kpool,
            tc.tile_pool(name="data", bufs=5) as dpool,
            tc.tile_pool(name="outp", bufs=6) as opool,
            tc.tile_pool(name="psd", bufs=4, space="PSUM") as psd_pool,
        ):
            # consts + mask on the SWDGE ring (idle until writes start)
            krl_t = cpool.tile([128, 4096], mybir.dt.uint8)
            nc.gpsimd.dma_start(krl_t[:], krl.ap())
            wbd_t = cpool.tile([128, 256], bf16)
            nc.gpsimd.dma_start(wbd_t[:], w_bd.ap())
            blhs_t = cpool.tile([128, 1024], bf16)
            nc.gpsimd.dma_start(blhs_t[:], blhs.ap())
            krl_bf = cpool.tile([128, 4096], bf16)
            nc.vector.tensor_copy(krl_bf[:], krl_t[:])

            # ---- all read DMAs up-front ----
            # tile 0 as four independent 2048-col chunks; chunk c1*2+q
            # holds f [(c1*4096 + q*2048) : +2048]
            chunks = []
            for c1 in range(2):
                for q in range(2):
                    ct = kpool.tile([128, 2048], bf16, tag="chunk")
                    chunks.append(ct)
            # issue order: (c1,q)=(0,0) sync, (1,0) scalar, (0,1) sync,
            # (1,1) scalar -- the first matmul pair needs (0,0)+(1,0)
            for c1, q, eng in ((0, 0, nc.sync), (1, 0, nc.scalar),
                               (0, 1, nc.sync), (1, 1, nc.scalar)):
                fsl = c1 * 4096 + q * 2048
                eng.dma_start(chunks[2 * c1 + q][:],
                              srl.ap()[0, 0][:, fsl:fsl + 2048])
            rhs = {}
            for t in range(1, 8):
                bb, w = divmod(t, 4)
                rhs[t] = dpool.tile([128, 8192], bf16, tag="rhs",
                                    name=f"rhs{t}")
                eng = nc.sync if t % 2 == 0 else nc.scalar
                eng.dma_start(rhs[t][:], srl.ap()[bb, w])

            # ---- compute + writes ----
            for t in range(8):
                bb, w = divmod(t, 4)
                out_t = opool.tile([128, 4096], bf16)
                for k in range(4):          # i_in pair (2k, 2k+1)
                    ps = psd_pool.tile([128, 1024], f32)
                    for u in range(2):      # i_in = 2k + u
                        i_in = 2 * k + u
                        for c1 in range(2):
                            if t == 0:
                                src = chunks[2 * c1 + (i_in // 4)]
                                fsl = (i_in % 4) * 512
                            else:
                                src = rhs[t]
                                fsl = (8 * c1 + i_in) * 512
                            nc.tensor.matmul(
                                ps[:, u * 512:u * 512 + 512],
                                wbd_t[:, c1 * 128:c1 * 128 + 128],
                                src[:, fsl:fsl + 512],
                                start=(c1 == 0), stop=False)
                        nc.tensor.matmul(
                            ps[:, u * 512:u * 512 + 512],
                            blhs_t[:, t * 128:t * 128 + 128],
                            krl_bf[:, i_in * 512:i_in * 512 + 512],
                            start=False, stop=True)
                    ceng = nc.vector if k % 2 == 0 else nc.scalar
                    if k % 2 == 0:
                        ceng.tensor_copy(
                            out_t[:, k * 1024:k * 1024 + 1024], ps[:])
                    else:
                        ceng.copy(
                            out_t[:, k * 1024:k * 1024 + 1024], ps[:])
                    if k % 2 == 1:          # half (i_in 4c..4c+4) done
                        c = k // 2
                        # early tiles write on the idle SWDGE ring; late
                        # tiles go on the HWDGE rings, whose reads have
                        # drained by then (keeps the SWDGE desc-gen
                        # serialization off the critical path)
                        if t < 4:
                            weng = nc.gpsimd
                        else:
                            weng = nc.sync if c == 0 else nc.scalar
                        weng.dma_start(
                            oview[bb, w, c],
                            out_t[:, c * 2048:c * 2048 + 2048])

    nc.compile()
    return nc


def _prep_consts(W, b):
    # c1-th accumulating matmul lhsT in w_bd[:, 128*c1:...]:
    # w_bd[8s+ih, 128*c1 + 16h + 2ih + c1] = W[h, s]; rest zero.
    w_bd = np.zeros((128, 256), dtype=np.float32)
    for c1 in range(2):
        for ih in range(8):
            for h in range(8):
                m = 16 * h + 2 * ih + c1
                w_bd[ih::8, 128 * c1 + m] = W[h, :]  # rows k = 8s+ih
    # bias-keep lhsT per tile t: blhs[16t+cd, 128t + 16h+cd] = b[h]
    blhs = np.zeros((128, 1024), dtype=np.float32)
    for t in range(8):
        for cd in range(16):
            for h in range(8):
                blhs[16 * t + cd, 128 * t + 16 * h + cd] = b[h]
    return (w_bd.astype(ml_dtypes.bfloat16),
            blhs.astype(ml_dtypes.bfloat16))


def _relayout(stacks, mask):
    keep = ~np.asarray(mask, bool)                       # [B, N, N]
    # pre-mask: masked pairs contribute exactly 0 to every h
    sm = np.asarray(stacks, np.float32) * keep[:, None, :, :]
    # srl[b, w, 8s+ih, il*512+j] = sm[b, s, 128w+16ih+il, j]
    srl = sm.reshape(B, S, 4, 8, 16, N)                  # b s w ih il j
    srl = srl.transpose(0, 2, 1, 3, 4, 5)                # b w s ih il j
    srl = np.ascontiguousarray(srl, dtype=ml_dtypes.bfloat16)
    srl = srl.reshape(B, 4, 128, 8192)
    # krl[16*(4b'+w)+cd, iin*512+j] = keep[b, 128w+8cd+iin, j], per core
    krl = keep.astype(np.uint8).reshape(B, 4, 16, 8, N)  # b w cd iin j
    krl = krl.reshape(NCORES, BPC * 4 * 16, 8 * N)       # core, 128, 4096
    return srl, krl


def kernel(stacks, mask, W, b):
    from concourse.bass_utils import run_bass_kernel_spmd

    if "nc" not in _CACHE:
        _CACHE["nc"] = _build()
    nc = _CACHE["nc"]

    srl, krl = _relayout(stacks, mask)
    w_bd, blhs = _prep_consts(np.asarray(W, np.float32),
                              np.asarray(b, np.float32))

    in_maps = []
    for c in range(NCORES):
        in_maps.append({
            "srl": srl[c * BPC:(c + 1) * BPC],
            "krl": krl[c],
            "w_bd": w_bd, "blhs": blhs,
        })

    res = run_bass_kernel_spmd(nc, in_maps, core_ids=list(range(NCORES)),
                               **_CACHE.get("run_kwargs", {}))
    _CACHE["last_result"] = res
    outs = [np.asarray(r["out"]) for r in res.results]
    return np.concatenate(outs, axis=0).astype(np.float32)
